# revision 1
# baseline (speedup 1.0000x reference)
"""Fused multi-head self-attention (degenerate seq-len-1) + LayerNorm for TRN2.

Math: with sequence length 1, softmax over the single key is exactly 1.0, so
attention output == v.  The whole module collapses to

    out = LayerNorm((x @ W_v.T + b_v) @ W_proj.T + b_proj) * gamma + beta
        = LayerNorm(x @ C.T + bias) * gamma + beta

with C = W_proj @ W_v and bias = W_proj @ b_v + b_proj (both batch-independent,
folded on the host).  The device kernel is a single [1024,4096]x[4096,4096]
matmul per core (batch data-parallel over 8 cores) fused with LayerNorm.
"""

import os
import sys

import numpy as np

if "/opt/trn_rl_repo" not in sys.path:
    sys.path.insert(0, "/opt/trn_rl_repo")

import ml_dtypes

P = 128              # SBUF partitions
DIM = 4096
B = 8192
NCORES = 8
BL = B // NCORES     # batch rows per core
BT = BL // P         # b tiles per core
KO = DIM // P        # contraction tiles
JC = 256             # moving free-dim chunk (output cols per matmul)
NJC = DIM // JC
EPS = 1e-5

_BUILD_CACHE = {}


def _build(apply_bias: bool, apply_affine: bool):
    key = (apply_bias, apply_affine)
    if key in _BUILD_CACHE:
        return _BUILD_CACHE[key]

    import concourse.mybir as mybir
    import concourse.tile as tile
    from concourse import bacc

    bf16 = mybir.dt.bfloat16
    f32 = mybir.dt.float32

    nc = bacc.Bacc("TRN2", target_bir_lowering=False, debug=False,
                   num_devices=NCORES)

    xt_d = nc.declare_dram_parameter("xt", [P, KO, BL], bf16, isOutput=False)
    ct_d = nc.declare_dram_parameter("ct", [NJC, P, KO, JC], bf16, isOutput=False)
    bias_d = nc.declare_dram_parameter("bias", [DIM], f32, isOutput=False)
    gamma_d = nc.declare_dram_parameter("gamma", [DIM], f32, isOutput=False)
    beta_d = nc.declare_dram_parameter("beta", [DIM], f32, isOutput=False)
    out_d = nc.declare_dram_parameter("out", [BT, P, NJC, JC], f32, isOutput=True)

    with tile.TileContext(nc) as tc:
        with tc.tile_pool(name="xpool", bufs=1) as xpool, \
             tc.tile_pool(name="wpool", bufs=2) as wpool, \
             tc.tile_pool(name="ypool", bufs=1) as ypool, \
             tc.tile_pool(name="spool", bufs=1) as spool, \
             tc.tile_pool(name="opool", bufs=3) as opool, \
             tc.tile_pool(name="small", bufs=4) as small, \
             tc.tile_pool(name="ppool", bufs=4, space="PSUM") as ppool:

            # Resident stationary operand x^T: [p, ko, b].  Loaded per-ko so
            # the first accumulation chain can start before the full 8 MiB
            # lands.
            xt_sb = xpool.tile([P, KO, BL], bf16)
            for ko in range(KO):
                nc.sync.dma_start(out=xt_sb[:, ko, :], in_=xt_d[:, ko, :])

            # y (pre-norm matmul result) stays resident in bf16.
            y_sb = ypool.tile([P, BT, NJC, JC], bf16)
            # Per-chunk bn_stats, aggregated per b-tile at the end.
            stats_sb = spool.tile([P, BT, NJC, 6], f32)

            bias_sb = None
            if apply_bias:
                bias_sb = spool.tile([P, NJC, JC], f32)
                nc.sync.dma_start(out=bias_sb,
                                  in_=bias_d.ap().to_broadcast([P, NJC, JC]))

            for jc in range(NJC):
                ct_sb = wpool.tile([P, KO, JC], bf16)
                nc.sync.dma_start(out=ct_sb, in_=ct_d[jc])
                for bt in range(BT):
                    ps = ppool.tile([P, JC], f32)
                    for ko in range(KO):
                        nc.tensor.matmul(
                            ps,
                            lhsT=xt_sb[:, ko, bt * P:(bt + 1) * P],
                            rhs=ct_sb[:, ko, :],
                            start=(ko == 0),
                            stop=(ko == KO - 1),
                        )
                    if apply_bias:
                        nc.vector.tensor_add(y_sb[:, bt, jc, :], ps,
                                             bias_sb[:, jc, :])
                        nc.vector.bn_stats(stats_sb[:, bt, jc, :],
                                           y_sb[:, bt, jc, :])
                    else:
                        # ACT evicts PSUM (cast to bf16); DVE reads the same
                        # PSUM tile for the LayerNorm statistics.
                        nc.scalar.activation(y_sb[:, bt, jc, :], ps,
                                             mybir.ActivationFunctionType.Copy)
                        nc.vector.bn_stats(stats_sb[:, bt, jc, :], ps)

            eps_sb = small.tile([P, 1], f32)
            nc.vector.memset(eps_sb, EPS)

            gamma_sb = beta_sb = None
            if apply_affine:
                gamma_sb = spool.tile([P, NJC, JC], f32)
                nc.sync.dma_start(out=gamma_sb,
                                  in_=gamma_d.ap().to_broadcast([P, NJC, JC]))
                beta_sb = spool.tile([P, NJC, JC], f32)
                nc.sync.dma_start(out=beta_sb,
                                  in_=beta_d.ap().to_broadcast([P, NJC, JC]))

            JL = 4  # output chunks of JL*JC = 1024 columns
            for bt in range(BT):
                mv = small.tile([P, 2], f32)
                nc.vector.bn_aggr(mv, stats_sb[:, bt, :, :])
                std = small.tile([P, 1], f32)
                nc.scalar.activation(std, mv[:, 1:2],
                                     mybir.ActivationFunctionType.Sqrt,
                                     bias=eps_sb)
                rstd = small.tile([P, 1], f32)
                nc.vector.reciprocal(rstd, std)
                for j0 in range(0, NJC, JL):
                    o = opool.tile([P, JL, JC], f32)
                    nc.vector.tensor_scalar(
                        o, y_sb[:, bt, j0:j0 + JL, :],
                        scalar1=mv[:, 0:1], scalar2=rstd,
                        op0=mybir.AluOpType.subtract,
                        op1=mybir.AluOpType.mult,
                    )
                    if apply_affine:
                        nc.vector.tensor_mul(o, o, gamma_sb[:, j0:j0 + JL, :])
                        nc.vector.tensor_add(o, o, beta_sb[:, j0:j0 + JL, :])
                    nc.sync.dma_start(out=out_d[bt, :, j0:j0 + JL, :], in_=o)

    nc.compile()
    _BUILD_CACHE[key] = nc
    return nc


def kernel(x, W_qkv, b_qkv, W_proj, b_proj, gamma, beta):
    from concourse.bass_utils import run_bass_kernel_spmd

    x = np.asarray(x, dtype=np.float32)
    W_qkv = np.asarray(W_qkv, dtype=np.float32)
    b_qkv = np.asarray(b_qkv, dtype=np.float32)
    W_proj = np.asarray(W_proj, dtype=np.float32)
    b_proj = np.asarray(b_proj, dtype=np.float32)
    gamma = np.asarray(gamma, dtype=np.float32)
    beta = np.asarray(beta, dtype=np.float32)

    # Fold the two projections (q/k are dead: seq len 1 => attention == v).
    W_v = W_qkv[2 * DIM:3 * DIM, :]
    C = W_proj @ W_v                          # [j, k]
    bias_total = W_proj @ b_qkv[2 * DIM:] + b_proj

    # C^T tiled for streaming: ct[jc, p, ko, jl] = C[jc*JC+jl, ko*P+p]
    Ct = np.ascontiguousarray(
        C.T.reshape(KO, P, NJC, JC).transpose(2, 1, 0, 3)
    ).astype(ml_dtypes.bfloat16)

    apply_bias = bool(np.any(bias_total))
    apply_affine = not (np.all(gamma == 1.0) and np.all(beta == 0.0))

    nc = _build(apply_bias, apply_affine)

    in_maps = []
    for i in range(NCORES):
        xs = x[i * BL:(i + 1) * BL]           # [BL, DIM]
        xt = np.ascontiguousarray(
            xs.T.reshape(KO, P, BL).transpose(1, 0, 2)
        ).astype(ml_dtypes.bfloat16)          # [p, ko, b]
        in_maps.append({
            "xt": xt,
            "ct": Ct,
            "bias": bias_total,
            "gamma": gamma,
            "beta": beta,
        })

    trace = bool(int(os.environ.get("KERNEL_TRACE", "0")))
    res = run_bass_kernel_spmd(nc, in_maps, core_ids=list(range(NCORES)),
                               trace=trace)
    if trace:
        kernel.last_exec_time_ns = res.exec_time_ns
        kernel.last_results = res

    out = np.concatenate(
        [r["out"].reshape(BL, DIM) for r in res.results], axis=0
    )
    return out


# revision 6
# speedup vs baseline: 1.0391x; 1.0391x over previous
"""Fused multi-head self-attention (degenerate seq-len-1) + LayerNorm for TRN2.

Math: with sequence length 1, softmax over the single key is exactly 1.0, so
attention output == v.  The whole module collapses to

    out = LayerNorm((x @ W_v.T + b_v) @ W_proj.T + b_proj) * gamma + beta
        = LayerNorm(x @ C.T + bias) * gamma + beta

with C = W_proj @ W_v and bias = W_proj @ b_v + b_proj (both batch-independent,
folded on the host).  The device kernel is a single [1024,4096]x[4096,4096]
matmul per core (batch data-parallel over 8 cores) fused with LayerNorm.
"""

import os
import sys

import numpy as np

if "/opt/trn_rl_repo" not in sys.path:
    sys.path.insert(0, "/opt/trn_rl_repo")

import ml_dtypes

P = 128              # SBUF partitions
DIM = 4096
B = 8192
NCORES = 8
BL = B // NCORES     # batch rows per core
BT = BL // P         # b tiles per core
KO = DIM // P        # contraction tiles
JC = 256             # moving free-dim chunk (output cols per matmul)
NJC = DIM // JC
EPS = 1e-5

_BUILD_CACHE = {}


def _build(apply_bias: bool, apply_affine: bool):
    key = (apply_bias, apply_affine)
    if key in _BUILD_CACHE:
        return _BUILD_CACHE[key]

    import concourse.mybir as mybir
    import concourse.tile as tile
    from concourse import bacc

    bf16 = mybir.dt.bfloat16
    f32 = mybir.dt.float32

    nc = bacc.Bacc("TRN2", target_bir_lowering=False, debug=False,
                   num_devices=NCORES)

    xt_d = nc.declare_dram_parameter("xt", [BT, P, KO, P], bf16, isOutput=False)
    ct_d = nc.declare_dram_parameter("ct", [NJC, P, KO, JC], bf16, isOutput=False)
    bias_d = nc.declare_dram_parameter("bias", [DIM], f32, isOutput=False)
    gamma_d = nc.declare_dram_parameter("gamma", [DIM], f32, isOutput=False)
    beta_d = nc.declare_dram_parameter("beta", [DIM], f32, isOutput=False)
    out_d = nc.declare_dram_parameter("out", [BT, P, NJC, JC], f32, isOutput=True)

    with tile.TileContext(nc) as tc:
        with tc.tile_pool(name="xpool", bufs=1) as xpool, \
             tc.tile_pool(name="wpool", bufs=2) as wpool, \
             tc.tile_pool(name="ypool", bufs=1) as ypool, \
             tc.tile_pool(name="spool", bufs=1) as spool, \
             tc.tile_pool(name="opool", bufs=3) as opool, \
             tc.tile_pool(name="small", bufs=4) as small, \
             tc.tile_pool(name="ppool", bufs=4, space="PSUM") as ppool:

            # Prefetch C chunk 0, then x b-tile 0, then C chunk 1, then the
            # remaining x b-tiles — the PE's first accumulation group needs
            # only ct[0] + xt[bt0] (3 MiB), not the full 8 MiB of x.
            ct_tiles = {}
            xt_sb = xpool.tile([P, BT, KO, P], bf16)

            ct_tiles[0] = wpool.tile([P, KO, JC], bf16, name="ct_sb", tag="ct")
            nc.sync.dma_start(out=ct_tiles[0], in_=ct_d[0])
            nc.sync.dma_start(out=xt_sb[:, 0], in_=xt_d[0])
            ct_tiles[1] = wpool.tile([P, KO, JC], bf16, name="ct_sb", tag="ct")
            nc.sync.dma_start(out=ct_tiles[1], in_=ct_d[1])
            for bt in range(1, BT):
                nc.sync.dma_start(out=xt_sb[:, bt], in_=xt_d[bt])

            # y (pre-norm matmul result) stays resident in bf16.
            y_sb = ypool.tile([P, BT, NJC, JC], bf16)
            # Per-chunk bn_stats, aggregated per b-tile at the end.
            stats_sb = spool.tile([P, BT, NJC, 6], f32)

            eps_sb = small.tile([P, 1], f32)
            nc.vector.memset(eps_sb, EPS)

            bias_sb = None
            if apply_bias:
                bias_sb = spool.tile([P, NJC, JC], f32)
                nc.sync.dma_start(out=bias_sb,
                                  in_=bias_d.ap().to_broadcast([P, NJC, JC]))

            gamma_sb = beta_sb = None
            if apply_affine:
                gamma_sb = spool.tile([P, NJC, JC], f32)
                nc.sync.dma_start(out=gamma_sb,
                                  in_=gamma_d.ap().to_broadcast([P, NJC, JC]))
                beta_sb = spool.tile([P, NJC, JC], f32)
                nc.sync.dma_start(out=beta_sb,
                                  in_=beta_d.ap().to_broadcast([P, NJC, JC]))

            JL = 4  # LayerNorm-apply chunks of JL*JC = 1024 columns

            def layernorm_apply(bt):
                """Aggregate stats and write the normalized b-tile."""
                mv = small.tile([P, 2], f32)
                nc.vector.bn_aggr(mv, stats_sb[:, bt, :, :])
                std = small.tile([P, 1], f32)
                nc.scalar.activation(std, mv[:, 1:2],
                                     mybir.ActivationFunctionType.Sqrt,
                                     bias=eps_sb)
                rstd = small.tile([P, 1], f32)
                nc.vector.reciprocal(rstd, std)
                for j0 in range(0, NJC, JL):
                    o = opool.tile([P, JL, JC], f32)
                    nc.vector.tensor_scalar(
                        o, y_sb[:, bt, j0:j0 + JL, :],
                        scalar1=mv[:, 0:1], scalar2=rstd,
                        op0=mybir.AluOpType.subtract,
                        op1=mybir.AluOpType.mult,
                    )
                    if apply_affine:
                        nc.vector.tensor_mul(o, o, gamma_sb[:, j0:j0 + JL, :])
                        nc.vector.tensor_add(o, o, beta_sb[:, j0:j0 + JL, :])
                    nc.sync.dma_start(out=out_d[bt, :, j0:j0 + JL, :], in_=o)

            for jc in range(NJC):
                ct_sb = ct_tiles.pop(jc)
                if jc + 2 < NJC:
                    ct_tiles[jc + 2] = wpool.tile([P, KO, JC], bf16,
                                                  name="ct_sb", tag="ct")
                    nc.sync.dma_start(out=ct_tiles[jc + 2], in_=ct_d[jc + 2])
                for bt in range(BT):
                    ps = ppool.tile([P, JC], f32)
                    for ko in range(KO):
                        nc.tensor.matmul(
                            ps,
                            lhsT=xt_sb[:, bt, ko, :],
                            rhs=ct_sb[:, ko, :],
                            start=(ko == 0),
                            stop=(ko == KO - 1),
                        )
                    if apply_bias:
                        nc.vector.tensor_add(y_sb[:, bt, jc, :], ps,
                                             bias_sb[:, jc, :])
                        nc.vector.bn_stats(stats_sb[:, bt, jc, :],
                                           y_sb[:, bt, jc, :])
                    else:
                        # ACT evicts PSUM (cast to bf16); DVE reads the same
                        # PSUM tile for the LayerNorm statistics.
                        nc.scalar.activation(y_sb[:, bt, jc, :], ps,
                                             mybir.ActivationFunctionType.Copy)
                        nc.vector.bn_stats(stats_sb[:, bt, jc, :], ps)
                    if jc == NJC - 1:
                        # Interleave the LayerNorm epilogue with the
                        # remaining b-tiles' matmuls.
                        layernorm_apply(bt)

    nc.compile()
    _BUILD_CACHE[key] = nc
    return nc


def kernel(x, W_qkv, b_qkv, W_proj, b_proj, gamma, beta):
    from concourse.bass_utils import run_bass_kernel_spmd

    x = np.asarray(x, dtype=np.float32)
    W_qkv = np.asarray(W_qkv, dtype=np.float32)
    b_qkv = np.asarray(b_qkv, dtype=np.float32)
    W_proj = np.asarray(W_proj, dtype=np.float32)
    b_proj = np.asarray(b_proj, dtype=np.float32)
    gamma = np.asarray(gamma, dtype=np.float32)
    beta = np.asarray(beta, dtype=np.float32)

    # Fold the two projections (q/k are dead: seq len 1 => attention == v).
    W_v = W_qkv[2 * DIM:3 * DIM, :]
    C = W_proj @ W_v                          # [j, k]
    bias_total = W_proj @ b_qkv[2 * DIM:] + b_proj

    # C^T tiled for streaming: ct[jc, p, ko, jl] = C[jc*JC+jl, ko*P+p]
    Ct = np.ascontiguousarray(
        C.T.reshape(KO, P, NJC, JC).transpose(2, 1, 0, 3)
    ).astype(ml_dtypes.bfloat16)

    apply_bias = bool(np.any(bias_total))
    apply_affine = not (np.all(gamma == 1.0) and np.all(beta == 0.0))

    nc = _build(apply_bias, apply_affine)

    in_maps = []
    for i in range(NCORES):
        xs = x[i * BL:(i + 1) * BL]           # [BL, DIM]
        # xt[bt, p, ko, b'] = xs[bt*P + b', ko*P + p]
        xt = np.ascontiguousarray(
            xs.T.reshape(KO, P, BT, P).transpose(2, 1, 0, 3)
        ).astype(ml_dtypes.bfloat16)
        in_maps.append({
            "xt": xt,
            "ct": Ct,
            "bias": bias_total,
            "gamma": gamma,
            "beta": beta,
        })

    trace = bool(int(os.environ.get("KERNEL_TRACE", "0")))
    res = run_bass_kernel_spmd(nc, in_maps, core_ids=list(range(NCORES)),
                               trace=trace)
    if trace:
        kernel.last_exec_time_ns = res.exec_time_ns
        kernel.last_results = res

    out = np.concatenate(
        [r["out"].reshape(BL, DIM) for r in res.results], axis=0
    )
    return out
